# revision 1
# baseline (speedup 1.0000x reference)
"""CascadedGroupAttention Trainium2 kernel.

Data-parallel over batch: B=512 split as 64 samples x 8 cores. Inside each
core a fully fused per-head cascade runs phase-major over sample blocks.

Key restructurings vs the reference:
  - qkv BN affine folded into matmul weights; bias applied via a ones-row
    appended to the feat operand (K=65).
  - softmax 1/sqrt(d) scale folded into the k weights.
  - depthwise 5x5 conv computed on the tensor engine as 25 PSUM-accumulated
    matmuls with [k;q]-interleaved diagonal weight matrices and
    edge-trimmed access patterns (no padding, no im2col).
  - attention computed transposed (P^T = K^T Q + ab^T) so softmax needs no
    transposes: raw exp is safe (logits bounded ~[-9, 10]), the denominator
    comes from a ones-column appended to v^T in the AV matmul, and the
    1/denom broadcast across partitions is done by gpsimd.
  - relative-position bias added by an identity-weight matmul accumulating
    onto the QK PSUM bank (ab is symmetric, so ab^T = ab).
  - relu'd head outputs collected in bf16; the output projection runs in
    bf16 with its BN affine folded into weights/final eviction bias.
"""

import os
import sys

import numpy as np

sys.path.insert(0, "/opt/trn_rl_repo")

import concourse.bass as bass  # noqa: E402
from concourse import bacc  # noqa: E402
import concourse.mybir as mybir  # noqa: E402
from concourse.masks import make_identity  # noqa: E402
from concourse.tile import TileContext  # noqa: E402

F32 = mybir.dt.float32
BF16 = mybir.dt.bfloat16

NHEADS = 4
KD = 16          # key dim
DV = 64          # per-head value dim
CH = 64          # per-head input channels (dim // heads)
RES = 14
N = RES * RES    # 196 tokens
DIM = 256
BATCH = 512
NCORES = 8
SPC = BATCH // NCORES   # samples per core = 64
BLK = 16                # samples per pipeline block
SCALE = KD ** -0.5

# tap order: center first so the first conv matmul covers the full output
# region (start=True then has full has_written coverage for accumulation)
TAPS = [(0, 0)] + [
    (dr, dc) for dr in range(-2, 3) for dc in range(-2, 3) if (dr, dc) != (0, 0)
]


def _prep_host(inp):
    """Fold BN affines into weights and build hardware-layout arrays."""
    qkv_w = np.asarray(inp["qkv_w"], np.float32)
    qkv_scale = np.asarray(inp["qkv_scale"], np.float32)
    qkv_bias = np.asarray(inp["qkv_bias"], np.float32)
    dw_w = np.asarray(inp["dw_w"], np.float32)
    dw_scale = np.asarray(inp["dw_scale"], np.float32)
    dw_bias = np.asarray(inp["dw_bias"], np.float32)
    proj_w = np.asarray(inp["proj_w"], np.float32)
    proj_scale = np.asarray(inp["proj_scale"], np.float32)
    proj_bias = np.asarray(inp["proj_bias"], np.float32)
    ab_full = np.asarray(inp["attention_biases"], np.float32)[
        :, np.asarray(inp["bias_idxs"])
    ]  # [4, 196, 196], symmetric in (n, m)

    # k and q each get an M=32 weight tile (cols 16:32 zero) so the 4-sample
    # col-tiled matmuls write full 32-row blocks (no uninitialized PSUM rows)
    w_k = np.zeros((NHEADS, 65, 32), np.float32)
    w_q = np.zeros((NHEADS, 65, 32), np.float32)
    w_v = np.zeros((NHEADS, 65, 64), np.float32)
    conv_diag = np.zeros((NHEADS, 25, 128, 128), np.float32)
    dwb_pat = np.zeros((NHEADS, 128, 1), np.float32)
    for i in range(NHEADS):
        for j in range(KD):
            w_k[i, :CH, j] = qkv_w[i, KD + j] * qkv_scale[i, KD + j] * SCALE
            w_k[i, CH, j] = qkv_bias[i, KD + j] * SCALE
            w_q[i, :CH, j] = qkv_w[i, j] * qkv_scale[i, j]
            w_q[i, CH, j] = qkv_bias[i, j]
        for d in range(DV):
            w_v[i, :CH, d] = qkv_w[i, 2 * KD + d] * qkv_scale[i, 2 * KD + d]
            w_v[i, CH, d] = qkv_bias[i, 2 * KD + d]
        for t, (dr, dc) in enumerate(TAPS):
            for p in range(128):
                c = p % 32
                if c < 16:  # q channels sit in rows 32j..32j+15
                    conv_diag[i, t, p, p] = dw_w[i, c, dr + 2, dc + 2] * dw_scale[i, c]
        for p in range(128):
            c = p % 32
            if c < 16:
                dwb_pat[i, p, 0] = dw_bias[i, c]

    proj_wT = np.ascontiguousarray(
        (proj_w * proj_scale[:, None]).T
    )  # [cat_c, out_o]
    pw_bf = proj_wT.astype(np.dtype("bfloat16") if False else np.float32)
    # ship proj weights as bf16 via uint16 view workaround-free: use ml_dtypes
    import ml_dtypes

    pw_bf = proj_wT.astype(ml_dtypes.bfloat16)
    pb = np.ascontiguousarray(proj_bias.reshape(2, 128, 1).astype(np.float32))

    return {
        "w_k": w_k,
        "w_q": w_q,
        "w_v": w_v,
        "conv_diag": conv_diag,
        "dwb_pat": dwb_pat,
        "ab": np.ascontiguousarray(ab_full),
        "proj_wT": pw_bf,
        "proj_b": pb,
    }


def build_bass(spc=SPC, blk=BLK):
    nc = bacc.Bacc(None, target_bir_lowering=False)

    x_d = nc.declare_dram_parameter("x", [spc, DIM, N], F32, isOutput=False)
    wk_d = nc.declare_dram_parameter("w_k", [NHEADS, 65, 32], F32, isOutput=False)
    wq_d = nc.declare_dram_parameter("w_q", [NHEADS, 65, 32], F32, isOutput=False)
    wv_d = nc.declare_dram_parameter("w_v", [NHEADS, 65, 64], F32, isOutput=False)
    cdiag_d = nc.declare_dram_parameter(
        "conv_diag", [NHEADS, 25, 128, 128], F32, isOutput=False
    )
    dwb_d = nc.declare_dram_parameter("dwb_pat", [NHEADS, 128, 1], F32, isOutput=False)
    ab_d = nc.declare_dram_parameter("ab", [NHEADS, N, N], F32, isOutput=False)
    pw_d = nc.declare_dram_parameter("proj_wT", [DIM, DIM], BF16, isOutput=False)
    pb_d = nc.declare_dram_parameter("proj_b", [2, 128, 1], F32, isOutput=False)
    out_d = nc.declare_dram_parameter("out", [spc, DIM, N], F32, isOutput=True)
    dbg = os.environ.get("DEBUG_TAPS", "0") == "1"
    if dbg:
        dbg_feat = nc.declare_dram_parameter("dbg_feat", [65, N], F32, isOutput=True)
        dbg_k = nc.declare_dram_parameter("dbg_k", [128, N], F32, isOutput=True)
        dbg_qf = nc.declare_dram_parameter("dbg_qf", [128, N], F32, isOutput=True)
        dbg_vT = nc.declare_dram_parameter("dbg_vT", [128, 130], F32, isOutput=True)
        dbg_eP0 = nc.declare_dram_parameter("dbg_eP0", [128, 392], F32, isOutput=True)
        dbg_rcp = nc.declare_dram_parameter("dbg_rcp", [1, 784], F32, isOutput=True)
        dbg_bc = nc.declare_dram_parameter("dbg_bc", [64, N], F32, isOutput=True)
        dbg_g8 = nc.declare_dram_parameter("dbg_g8", [128, N], F32, isOutput=True)
        dbg_den = nc.declare_dram_parameter("dbg_den", [1, 784], F32, isOutput=True)

    nblk = spc // blk
    ngrp = blk // 4    # kq groups of 4 samples
    npair = blk // 2

    with TileContext(nc) as tc:
        with (
            tc.tile_pool(name="const", bufs=1) as constp,
            tc.tile_pool(name="persist", bufs=1) as persist,
            tc.tile_pool(name="work", bufs=3) as work,
            tc.tile_pool(name="outp", bufs=4) as outp,
        ):
            # ---- constants ----
            ident = constp.tile([128, 128], F32, name="ident")
            make_identity(nc, ident)
            wk_sb = []
            wq_sb = []
            wv_sb = []
            dwb_sb = []
            ab0_sb = []
            ab1_sb = []
            for i in range(NHEADS):
                t = constp.tile([65, 32], F32, name=f"wk{i}", tag=f"wk{i}")
                nc.sync.dma_start(out=t, in_=wk_d[i])
                wk_sb.append(t)
                t = constp.tile([65, 32], F32, name=f"wq{i}", tag=f"wq{i}")
                nc.sync.dma_start(out=t, in_=wq_d[i])
                wq_sb.append(t)
                t = constp.tile([65, 64], F32, name=f"wv{i}", tag=f"wv{i}")
                nc.sync.dma_start(out=t, in_=wv_d[i])
                wv_sb.append(t)
                t = constp.tile([128, 1], F32, name=f"dwb{i}", tag=f"dwb{i}")
                nc.sync.dma_start(out=t, in_=dwb_d[i])
                dwb_sb.append(t)
                t = constp.tile([128, N], F32, name=f"ab0_{i}", tag=f"ab0_{i}")
                nc.sync.dma_start(out=t, in_=ab_d[i, 0:128, :])
                ab0_sb.append(t)
                t = constp.tile([68, N], F32, name=f"ab1_{i}", tag=f"ab1_{i}")
                nc.sync.dma_start(out=t, in_=ab_d[i, 128:196, :])
                ab1_sb.append(t)
            cdg_sb = []
            for i in range(NHEADS):
                t = constp.tile([128, 25, 128], F32, name=f"cdg{i}", tag=f"cdg{i}")
                nc.sync.dma_start(out=t, in_=cdiag_d[i].rearrange("t p c -> p t c"))
                cdg_sb.append(t)
            pw0 = constp.tile([128, DIM], BF16, name="pw0")
            nc.sync.dma_start(out=pw0, in_=pw_d[0:128, :])
            pw1 = constp.tile([128, DIM], BF16, name="pw1")
            nc.sync.dma_start(out=pw1, in_=pw_d[128:256, :])
            pb0 = constp.tile([128, 1], F32, name="pb0")
            nc.sync.dma_start(out=pb0, in_=pb_d[0])
            pb1 = constp.tile([128, 1], F32, name="pb1")
            nc.sync.dma_start(out=pb1, in_=pb_d[1])

            for b in range(min(nblk, int(os.environ.get('MAXBLK', '99')))):
                s0 = b * blk
                # ---- per-block persistent tiles ----
                feat = []
                for sl in range(blk):
                    t = persist.tile(
                        [65, N], F32, name=f"feat{b}_{sl}", tag=f"feat{sl}"
                    )
                    nc.sync.dma_start(out=t[0:CH, :], in_=x_d[s0 + sl, 0:CH, :])
                    nc.gpsimd.memset(t[CH : CH + 1, :], 1.0)
                    if dbg and b == 0 and sl == 0:
                        nc.sync.dma_start(out=dbg_feat[:, :], in_=t)
                    feat.append(t)
                rcat01 = []
                rcat23 = []
                for p in range(npair):
                    rcat01.append(
                        persist.tile(
                            [128, 392], BF16, name=f"rA{b}_{p}", tag=f"rA{p}"
                        )
                    )
                    rcat23.append(
                        persist.tile(
                            [128, 392], BF16, name=f"rB{b}_{p}", tag=f"rB{p}"
                        )
                    )

                for i in range(NHEADS):
                    # prefetch next head's x chunk
                    x_sb = None
                    if i < NHEADS - 1:
                        x_sb = []
                        for sl in range(blk):
                            t = work.tile(
                                [CH, N],
                                F32,
                                name=f"x{b}_{i}_{sl}",
                                tag=f"x{sl}",
                                bufs=2,
                            )
                            nc.sync.dma_start(
                                out=t, in_=x_d[s0 + sl, (i + 1) * CH : (i + 2) * CH, :]
                            )
                            x_sb.append(t)

                    # ================= phase A =================
                    kf = []   # per-group k tiles  (k at rows 32j..32j+16)
                    qf = []   # per-group q tiles  (conv+gelu+residual applied)
                    vT = []
                    with tc.tile_pool(name=f"psA{b}_{i}", bufs=1, space="PSUM") as pA:
                        # k/q matmuls (col-tiled M=32, 4 samples per bank)
                        k_ps = []
                        q_ps = []
                        for g in range(ngrp):
                            kp = pA.tile([128, 512], F32, name=f"kp{g}", tag="kp", bufs=2)
                            qp = pA.tile([128, 512], F32, name=f"qp{g}", tag="qp", bufs=2)
                            for j in range(4):
                                nc.tensor.matmul(
                                    kp[32 * j : 32 * j + 32, 0:N],
                                    wk_sb[i],
                                    feat[4 * g + j],
                                    start=True,
                                    stop=True,
                                    tile_position=(0, 32 * j),
                                )
                                nc.tensor.matmul(
                                    qp[32 * j : 32 * j + 32, 0:N],
                                    wq_sb[i],
                                    feat[4 * g + j],
                                    start=True,
                                    stop=True,
                                    tile_position=(0, 32 * j),
                                )
                            k_ps.append(kp)
                            q_ps.append(qp)
                        # v matmuls (col-tiled M=32 x2, 2 samples per bank)
                        v_ps = []
                        for p in range(npair):
                            vp = pA.tile([128, 512], F32, name=f"vp{p}", tag="vp", bufs=1)
                            for j2 in range(2):
                                sl = 2 * p + j2
                                for half in range(2):
                                    nc.tensor.matmul(
                                        vp[
                                            64 * j2 + 32 * half : 64 * j2
                                            + 32 * half
                                            + 32,
                                            0:N,
                                        ],
                                        wv_sb[i][:, 32 * half : 32 * half + 32],
                                        feat[sl],
                                        start=True,
                                        stop=True,
                                        tile_position=(0, 64 * j2 + 32 * half),
                                    )
                            v_ps.append(vp)

                        # evict k/q, conv, gelu, residual
                        for g in range(ngrp):
                            k_t = persist.tile(
                                [128, N], F32, name=f"kT{b}_{i}_{g}", tag=f"kT{g}"
                            )
                            nc.scalar.copy(k_t, k_ps[g][:, 0:N])
                            if dbg and b == 0 and i == 0 and g == 0:
                                nc.sync.dma_start(out=dbg_k[:, :], in_=k_t)
                            kf.append(k_t)
                            qpad = work.tile([128, 324], F32, name=f"qpad{g}",
                                             tag="qpad")
                            nc.gpsimd.memset(qpad, 0.0)
                            qp3 = qpad.rearrange("p (r c) -> p r c", c=18)
                            nc.scalar.copy(
                                qp3[:, 2:16, 2:16],
                                q_ps[g][:, 0:N].rearrange("p (r c) -> p r c", c=RES),
                            )
                            dqp = pA.tile([128, N], F32, name=f"dqp{g}", tag="dqp",
                                          bufs=2)
                            for t, (dr, dc) in enumerate(TAPS):
                                nc.tensor.matmul(
                                    dqp[:, 0:N],
                                    cdg_sb[i][:, t, :],
                                    qp3[:, 2 + dr : 16 + dr, 2 + dc : 16 + dc],
                                    start=(t == 0),
                                    stop=(t == len(TAPS) - 1),
                                )
                            g8 = work.tile([128, N], F32, name=f"g8{g}", tag="g8")
                            nc.scalar.activation(
                                g8, dqp, mybir.ActivationFunctionType.Gelu,
                                bias=dwb_sb[i], scale=1.0,
                            )
                            qf_t = persist.tile(
                                [128, N], F32, name=f"qf{b}_{i}_{g}", tag=f"qf{g}"
                            )
                            nc.vector.tensor_add(
                                qf_t.rearrange("p (r c) -> p r c", c=RES),
                                g8.rearrange("p (r c) -> p r c", c=RES),
                                qp3[:, 2:16, 2:16],
                            )
                            if dbg and b == 0 and i == 0 and g == 0:
                                nc.sync.dma_start(out=dbg_qf[:, :], in_=qf_t)
                                nc.sync.dma_start(out=dbg_g8[:, :], in_=g8)
                            qf.append(qf_t)

                        # v evict + transposes
                        for p in range(npair):
                            v_sb = work.tile([128, N], F32, name=f"vs{p}", tag="vs")
                            nc.vector.tensor_copy(v_sb, v_ps[p][:, 0:N])
                            for j2 in range(2):
                                sl = 2 * p + j2
                                vTp = pA.tile([128, 128], F32, name=f"vTp{sl}",
                                              tag="vTp", bufs=1)
                                nc.tensor.transpose(
                                    vTp[0:128, 0:64],
                                    v_sb[64 * j2 : 64 * j2 + 64, 0:128],
                                    ident[64 * j2 : 64 * j2 + 64,
                                          64 * j2 : 64 * j2 + 64],
                                )
                                nc.tensor.transpose(
                                    vTp[0:68, 64:128],
                                    v_sb[64 * j2 : 64 * j2 + 64, 128:196],
                                    ident[64 * j2 : 64 * j2 + 64,
                                          64 * j2 : 64 * j2 + 64],
                                )
                                vT_t = persist.tile(
                                    [128, 130], F32, name=f"vT{b}_{i}_{sl}",
                                    tag=f"vT{sl}",
                                )
                                nc.vector.tensor_copy(
                                    vT_t[:, 0:64], vTp[:, 0:64]
                                )
                                nc.vector.tensor_copy(
                                    vT_t[0:68, 65:129], vTp[0:68, 64:128]
                                )
                                nc.gpsimd.memset(vT_t[:, 64:65], 1.0)
                                nc.gpsimd.memset(vT_t[:, 129:130], 1.0)
                                if dbg and b == 0 and i == 0 and sl == 0:
                                    nc.sync.dma_start(out=dbg_vT[:, :], in_=vT_t)
                                vT.append(vT_t)

                    # ================= phase B =================
                    with tc.tile_pool(name=f"psB{b}_{i}", bufs=1, space="PSUM") as pB:
                        for q4 in range(blk // 4):  # 4 samples at a time
                            av = pB.tile([65, 2048], F32, name=f"av{q4}", tag="av")
                            for ph in range(2):  # pairs in this quad
                                pT0 = pB.tile(
                                    [128, 1024], F32, name=f"pT0_{ph}", tag="pT0"
                                )
                                pT1 = pB.tile(
                                    [68, 1024], F32, name=f"pT1_{ph}", tag="pT1"
                                )
                                for j2 in range(2):
                                    sl = 4 * q4 + 2 * ph + j2
                                    g, j = sl // 4, sl % 4
                                    kf_s = kf[g][32 * j : 32 * j + 16, :]
                                    qf_s = qf[g][32 * j : 32 * j + 16, 0:N]
                                    off = 512 * j2
                                    nc.tensor.matmul(
                                        pT0[:, off : off + 196],
                                        kf_s[:, 0:128],
                                        qf_s,
                                        start=True,
                                        stop=False,
                                        tile_position=(32 * j, 0),
                                    )
                                    nc.tensor.matmul(
                                        pT1[:, off : off + 196],
                                        kf_s[:, 128:196],
                                        qf_s,
                                        start=True,
                                        stop=False,
                                        tile_position=(32 * j, 0),
                                    )
                                    nc.tensor.matmul(
                                        pT0[:, off : off + 196],
                                        ident,
                                        ab0_sb[i],
                                        start=False,
                                        stop=True,
                                    )
                                    nc.tensor.matmul(
                                        pT1[:, off : off + 196],
                                        ident[0:68, 0:68],
                                        ab1_sb[i],
                                        start=False,
                                        stop=True,
                                    )
                                # exp (2 samples per op via bank-strided AP)
                                eP0 = work.tile(
                                    [128, 392], F32, name=f"eP0_{ph}", tag=f"eP0_{ph}"
                                )
                                nc.scalar.activation(
                                    eP0.rearrange("p (a c) -> p a c", a=2),
                                    pT0.rearrange("p (a c) -> p a c", a=2)[:, :, 0:196],
                                    mybir.ActivationFunctionType.Exp,
                                )
                                if dbg and b == 0 and i == 0 and q4 == 0 and ph == 0:
                                    nc.sync.dma_start(out=dbg_eP0[:, :], in_=eP0)
                                eP1 = work.tile(
                                    [68, 392], F32, name=f"eP1_{ph}", tag=f"eP1_{ph}"
                                )
                                nc.scalar.activation(
                                    eP1.rearrange("p (a c) -> p a c", a=2),
                                    pT1.rearrange("p (a c) -> p a c", a=2)[:, :, 0:196],
                                    mybir.ActivationFunctionType.Exp,
                                )
                                # AV with ones-column denominator (av rows 0:64 =
                                # out, row 64 = softmax denominator)
                                for j2 in range(2):
                                    sl = 4 * q4 + 2 * ph + j2
                                    u = 2 * ph + j2
                                    nc.tensor.matmul(
                                        av[:, 512 * u : 512 * u + 196],
                                        vT[sl][0:128, 0:65],
                                        eP0[:, 196 * j2 : 196 * j2 + 196],
                                        start=True,
                                        stop=False,
                                    )
                                    nc.tensor.matmul(
                                        av[:, 512 * u : 512 * u + 196],
                                        vT[sl][0:68, 65:130],
                                        eP1[:, 196 * j2 : 196 * j2 + 196],
                                        start=False,
                                        stop=True,
                                    )
                            # reciprocal of denominators (batched, 4 samples).
                            # custom-DVE ops give garbage on PSUM sources, so
                            # stage the denominator row through SBUF via ACT.
                            den4 = work.tile([1, 784], F32, name="den4", tag="den4")
                            nc.scalar.copy(
                                den4.rearrange("p (a c) -> p a c", a=4),
                                av[64:65, :].rearrange("p (a c) -> p a c", a=4)[
                                    :, :, 0:196
                                ],
                            )
                            rcp = work.tile([1, 784], F32, name="rcp", tag="rcp")
                            nc.vector.reciprocal_approx_fast(rcp, den4)
                            for u in range(4):
                                sl = 4 * q4 + u
                                bc = work.tile([64, N], F32, name="bc", tag="bc")
                                nc.gpsimd.partition_broadcast(
                                    bc, rcp[0:1, 196 * u : 196 * u + 196]
                                )
                                if dbg and b == 0 and i == 0 and q4 == 0 and u == 0:
                                    den_sb = work.tile([1, 784], F32, name="den_sb",
                                                       tag="den_sb")
                                    nc.vector.tensor_copy(
                                        den_sb.rearrange("p (a c) -> p a c", a=4),
                                        av[64:65, :].rearrange(
                                            "p (a c) -> p a c", a=4)[:, :, 0:196],
                                    )
                                    nc.sync.dma_start(out=dbg_den[:, :], in_=den_sb)
                                    nc.sync.dma_start(out=dbg_rcp[:, :], in_=rcp)
                                    nc.sync.dma_start(out=dbg_bc[:, :], in_=bc)
                                avs = av[0:64, 512 * u : 512 * u + 196]
                                rc = (rcat01 if i < 2 else rcat23)[sl // 2][
                                    64 * (i % 2) : 64 * (i % 2) + 64,
                                    196 * (sl % 2) : 196 * (sl % 2) + 196,
                                ]
                                if i < NHEADS - 1:
                                    nc.vector.tensor_mul(feat[sl][0:64, :], avs, bc)
                                    nc.gpsimd.tensor_scalar_max(
                                        rc, feat[sl][0:64, :], 0.0
                                    )
                                    nc.gpsimd.tensor_add(
                                        feat[sl][0:64, :], feat[sl][0:64, :], x_sb[sl]
                                    )
                                else:
                                    nc.vector.scalar_tensor_tensor(
                                        rc,
                                        avs,
                                        0.0,
                                        bc,
                                        op0=mybir.AluOpType.max,
                                        op1=mybir.AluOpType.mult,
                                    )

                # ---- projection + output ----
                with tc.tile_pool(name=f"psP{b}", bufs=2, space="PSUM") as pP:
                    for p in range(npair):
                        for m in range(2):
                            op = pP.tile([128, 392], F32, name=f"op{p}_{m}", tag=f"op{m}")
                            nc.tensor.matmul(
                                op,
                                pw0[:, 128 * m : 128 * m + 128],
                                rcat01[p],
                                start=True,
                                stop=False,
                            )
                            nc.tensor.matmul(
                                op,
                                pw1[:, 128 * m : 128 * m + 128],
                                rcat23[p],
                                start=False,
                                stop=True,
                            )
                            ob = outp.tile([128, 392], F32, name=f"ob{m}", tag=f"ob{m}")
                            if m == 0:
                                nc.scalar.activation(
                                    ob, op, mybir.ActivationFunctionType.Identity,
                                    bias=pb0, scale=1.0,
                                )
                            else:
                                nc.vector.tensor_scalar_add(ob, op, pb1)
                            nc.sync.dma_start(
                                out=out_d[
                                    s0 + 2 * p : s0 + 2 * p + 2,
                                    128 * m : 128 * m + 128,
                                    :,
                                ].rearrange("s o n -> o s n"),
                                in_=ob.rearrange("o (s n) -> o s n", s=2),
                            )
    nc.finalize()
    return nc


_CACHE = {}


def _get_nc():
    if "nc" not in _CACHE:
        _CACHE["nc"] = build_bass()
    return _CACHE["nc"]


def kernel(**inputs) -> np.ndarray:
    from concourse.bass_utils import run_bass_kernel_spmd

    host = _prep_host(inputs)
    x = np.asarray(inputs["x"], np.float32).reshape(BATCH, DIM, N)

    nc = _get_nc()
    in_maps = []
    for c in range(NCORES):
        m = {"x": np.ascontiguousarray(x[c * SPC : (c + 1) * SPC])}
        m.update(host)
        in_maps.append(m)
    res = run_bass_kernel_spmd(nc, in_maps, list(range(NCORES)))
    out = np.concatenate([r["out"] for r in res.results], axis=0)
    return out.reshape(BATCH, DIM, RES, RES).astype(np.float32)

